# revision 1
# baseline (speedup 1.0000x reference)
"""Block-sparse causal attention kernel for Trainium2 (8 NeuronCores).

Problem: B=2, T=2048, H=16, Dqk=Dv=128, fp16, BLOCK 64x64 block mask +
causal, softmax over keys.

Sharding: the 32 (b, h) pairs are split 4-per-core across 8 cores (data +
head parallel); no cross-core communication.

Per-core device algorithm (per (b,h) pair):
  - Inputs arrive as one contiguous [128, 6784] fp16 plane per pair,
    [K^T | Q^T | V | mask-table], with Q^T/K^T pre-transposed on the host
    and V laid out [t mod 128 -> partition, 16 k-tiles, d]. No device-side
    transposes (avoids the DMA xbar mode toggles); all DMAs ride the SP
    HWDGE queue (DMAs on a compute engine's queue would head-of-line block
    its dispatch). The first pair's load is split so K^T tile 0 + Q^T
    group 0 land first and the PE starts ~1us in.
  - Loop q-groups g (512 queries each), inner k-tile pairs (kt0, kt0+1):
      S^T[n,m] = K_kt @ Q_g^T  on PE (out [128 keys, 512 queries] psum)
      P^T = exp(S^T * 1/sqrt(d))  on ACT (fp16 to SBUF), no max-subtraction
      P^T *= blockmask (broadcast 0/1, DVE); diagonal chunk *= causal 0/1
      O^T[d,m] += V_kt^T @ P^T  on PE (accumulate over kt in psum)
      l[1,m]  += ones^T @ P^T   on PE, with the 4 denominator matmuls of
        each 4-k-tile quad packed onto disjoint 32-wide PE column groups
        (tile_position) so they stream concurrently (~4x cheaper); their
        partial rows land on psum partitions {0,32,64,96} and the host sums.
  - Readout: evacuate [O^T | l] (unnormalized fp32, one 2-bank psum tile)
    with one strided DVE copy per group into a per-pair staging plane and
    DMA out per group; the host does the final [d, t] -> [t, d] transpose
    fused with the 1/l softmax normalization.

The emission is software-pipelined with a lookahead of LOOKAHEAD steps: the
S^T matmuls (and exp/mask) of steps s+1..s+LA are issued before the O/l
matmuls of step s, so the PE streams through the exp/mask latency of the
ACT/DVE chain instead of stalling on it, including across group/pair
boundaries. PSUM: 3 double-bank S tiles + the combined [O | l] accumulator.

Causal suffix-trimming packs each step's two live slabs contiguously
([off0:512] and [512:512+w1]) so every step needs exactly one exp and one
mask-multiply; the host-built mask table mirrors this packing.

The block mask is applied multiplicatively after exp (scores are O(5), so
exp never overflows), which keeps the program identical across all cores:
the mask enters only as data (a per-(bh, step) 0/1 table plus one shared
in-block causal-triangle tile), so SPMD holds even though each core sees
different masks.
"""

import numpy as np

import concourse.bass as bass
import concourse.mybir as mybir
import concourse.tile as tile
from concourse import bacc

B, T, H, D = 2, 2048, 16, 128
BM = 64           # mask block size
NT = T // 128     # 16 k-tiles / q-tiles of 128
NG = 4            # q-groups of 512 queries
BH_PER_CORE = 4
N_CORES = 8
SCALE = float(1.0 / np.sqrt(D))

F16 = mybir.dt.float16
F32 = mybir.dt.float32

# step s enumerates (g, kt): for g in 0..3: for kt in 0..4g+3
STEP_OFF = [0, 4, 12, 24]
N_STEPS = 40

# combined input layout (columns of the per-bh [128, XCOLS] tile), ordered
# by first use so the first pair's chunked load streams in need-order:
# K^T tiles 0-3 and Q^T group 0 (first steps), the mask table (first mask),
# Q^T groups 1-3, V, then K^T tiles 4-15
XK03 = 0                    # K^T tiles 0-3   [128, 512]
XQ0 = XK03 + 512            # Q^T group 0     [128, 512]
XM = XQ0 + 512              # m8 mask table   [128, N_STEPS * 16]
XQ13 = XM + N_STEPS * 16    # Q^T groups 1-3  [128, 1536]
XV = XQ13 + 1536            # V               [128, NT, 128]
XK415 = XV + NT * 128       # K^T tiles 4-15  [128, 1536]
XCOLS = XK415 + 1536        # 6784


def kt_col(kt):
    return XK03 + 128 * kt if kt < 4 else XK415 + 128 * (kt - 4)


def qg_col(g):
    return XQ0 if g == 0 else XQ13 + 512 * (g - 1)

LOOKAHEAD = 3

# pack the 4 softmax-denominator matmuls of each 4-k-tile quad onto disjoint
# 32-wide PE column groups (tile_position) so they stream concurrently;
# their partial rows land on psum partitions {0,32,64,96} and the host sums
L_PACKED = True

# stagger the timing-loop's semaphore resets (no all-engine barrier on the
# back edge -> cross-iteration pipeline overlap)
import os as _os

STAGGERED = _os.environ.get("KERNEL_STAGGERED", "1") == "1"

# batch the o/l output DMAs per bh instead of per group (measured slower on
# HW: the big bh-end DMA stalls the serial descriptor chain; default off)
BATCH_OUT = _os.environ.get("KERNEL_BATCH_OUT", "0") == "1"


def build_program(loop_n=None):
    nc = bacc.Bacc("TRN2", target_bir_lowering=False, debug=False)

    x_d = nc.dram_tensor(
        "x", (BH_PER_CORE, 128, XCOLS), F16, kind="ExternalInput"
    )
    c01_d = nc.dram_tensor("c01", (128, 128), F16, kind="ExternalInput")
    # o is stored transposed ([d, t] per pair, fp32 unnormalized); host does
    # the final [d, t] -> [t, d] transpose fused with the 1/l normalization
    o_d = nc.dram_tensor("o", (BH_PER_CORE, D, T), F32, kind="ExternalOutput")
    NL = 4 if L_PACKED else 1
    l_d = nc.dram_tensor("l", (BH_PER_CORE, NL, T), F32, kind="ExternalOutput")

    with tile.TileContext(nc) as tc:
        with (
            tc.tile_pool(name="inp", bufs=4) as inp,
            tc.tile_pool(name="const", bufs=1) as cpool,
            tc.tile_pool(name="pt", bufs=6) as ppool,
            tc.tile_pool(name="outp", bufs=2) as opool,
            tc.tile_pool(name="sc", bufs=3, space="PSUM") as scpool,
            tc.tile_pool(name="ol", bufs=1, space="PSUM") as olpool,
        ):
            ones = cpool.tile([128, 1], F16)
            nc.vector.memset(ones[:, :], 1.0)
            # c01 is a constant (in-block causal triangle) — load it once,
            # outside the timing loop, like the ones memset
            c01 = cpool.tile([128, 128], F16)
            nc.sync.dma_start(c01[:, :], c01_d.ap()[:, :])

            if loop_n is not None:
                loop_cm = tc.For_i(
                    0,
                    loop_n,
                    1,
                    hint_engines=(
                        mybir.EngineType.PE,
                        mybir.EngineType.Activation,
                        mybir.EngineType.DVE,
                        mybir.EngineType.SP,
                        mybir.EngineType.Pool,
                    ),
                    staggered_reset=STAGGERED,
                )
                loop_cm.__enter__()

            tiles = []
            for bh in range(BH_PER_CORE):
                xt = inp.tile([128, XCOLS], F16, tag="xt")
                if bh == 0:
                    # chunked need-order startup load
                    for a, b in (
                        (XK03, XM),
                        (XM, XQ13),
                        (XQ13, XV),
                        (XV, XK415),
                        (XK415, XCOLS),
                    ):
                        nc.sync.dma_start(xt[:, a:b], x_d.ap()[bh][:, a:b])
                else:
                    # later pairs are prefetched well ahead; one DMA each
                    # minimizes HWDGE descriptor-generation occupancy
                    nc.sync.dma_start(xt[:, :], x_d.ap()[bh])
                tiles.append(xt)

            # software pipeline: pending holds the deferred PE consumer ops
            # (O/l matmuls of a step, group evacuations); they are emitted
            # LOOKAHEAD producer-steps later so the PE/DVE never stall on the
            # exp/mask chain of the step they consume
            pending = []

            def flush(limit=0):
                while len(pending) > limit:
                    pending.pop(0)()

            for bh in range(BH_PER_CORE):
                xt = tiles[bh]
                v = xt[:, XV : XV + NT * 128].rearrange("p (nt d) -> p nt d", d=128)
                m8 = xt[:, XM : XM + N_STEPS * 16]
                # per-bh output staging: o quarters contiguous in [0:T] so
                # the single per-bh DMA has one 8KB descriptor per partition;
                # l partial rows at [T:T+T]
                oln = opool.tile([128, 2 * T], F32, tag="oln")
                for g in range(NG):
                    nkt = 4 * g + 4
                    # combined accumulator: O^T in cols [0:512] (bank pair
                    # half A), l partials in cols [512:1024] (half B, on
                    # partitions {0,32,64,96} when packed); single 2-bank
                    # tile evacuated with one copy
                    ol = olpool.tile([128, 1024], F32)
                    prev_pt = None

                    for kt0 in range(0, nkt, 2):
                        # causal suffix-trim: columns below the diagonal chunk
                        # are fully masked; skip them. The h=1 slab is packed
                        # to start at column 512 regardless of its trim so
                        # the live region [off0 : 512+w1] stays contiguous
                        # (one exp, one mask-mul per step).
                        offs = [
                            max(0, kt0 + h - 4 * g) * 128 if kt0 + h > 4 * g else 0
                            for h in range(2)
                        ]
                        # per-half pt column ranges (packed layout)
                        rlo = [offs[0], 512]
                        rhi = [512, 512 + 512 - offs[1]]
                        sc = scpool.tile([128, 1024], F32)  # 2 psum banks
                        for h in range(2):
                            kt = kt0 + h
                            kc = kt_col(kt)
                            qc = qg_col(g)
                            nc.tensor.matmul(
                                sc[:, rlo[h] : rhi[h]],
                                lhsT=xt[:, kc : kc + 128],
                                rhs=xt[:, qc + offs[h] : qc + 512],
                                start=True,
                                stop=True,
                            )
                        pt = ppool.tile([128, 1024], F16)
                        s0 = STEP_OFF[g] + kt0
                        lo, hi = rlo[0], rhi[1]
                        nb = (hi - lo) // 64
                        nc.scalar.activation(
                            pt[:, lo:hi],
                            sc[:, lo:hi],
                            mybir.ActivationFunctionType.Exp,
                            scale=SCALE,
                        )
                        e0 = s0 * 16 + 2 * (lo // 64)
                        nc.vector.tensor_mul(
                            pt[:, lo:hi],
                            pt[:, lo:hi],
                            m8[:, e0 : e0 + 2 * nb]
                            .rearrange("p (j t) -> p j t", t=2)
                            .broadcast_to([128, nb, 2, 32])
                            .rearrange("p j t r -> p j r t"),
                        )
                        # in-chunk causal triangle on diagonal chunks; DVE
                        # is ~2.7x faster per op than gpsimd here and avoids
                        # the DVE<->Pool shared-SBUF-port contention, so this
                        # shortens the pt critical chain at group ends
                        for h in range(2):
                            kt = kt0 + h
                            if 4 * g <= kt <= 4 * g + 3:
                                c0 = rlo[h] + (kt - 4 * g) * 128 - offs[h]
                                nc.vector.tensor_mul(
                                    pt[:, c0 : c0 + 128],
                                    pt[:, c0 : c0 + 128],
                                    c01[:, :],
                                )

                        # deferred consumers of this step's pt: emitted
                        # LOOKAHEAD steps later so the PE stream runs ahead
                        prev = prev_pt
                        prev_pt = (pt, rlo, offs)

                        def make_consumer(
                            pt=pt,
                            prev=prev,
                            ol=ol,
                            offs=offs,
                            rlo=rlo,
                            rhi=rhi,
                            kt0=kt0,
                            nkt=nkt,
                            v=v,
                        ):
                            def consume():
                                for h in range(2):
                                    kt = kt0 + h
                                    nc.tensor.matmul(
                                        ol[:, offs[h] : 512],
                                        lhsT=v[:, kt, :],
                                        rhs=pt[:, rlo[h] : rhi[h]],
                                        start=(kt == 0),
                                        stop=(kt == nkt - 1),
                                    )
                                    if not L_PACKED or nkt == 4:
                                        # unpacked denominator (also for the
                                        # single-quad g=0 group, where packed
                                        # trims would leave unwritten holes)
                                        nc.tensor.matmul(
                                            ol[0:1, 512 + offs[h] :],
                                            lhsT=ones[:, :],
                                            rhs=pt[:, rlo[h] : rhi[h]],
                                            start=(kt == 0),
                                            stop=(kt == nkt - 1),
                                        )
                                if L_PACKED and nkt > 4 and kt0 % 4 == 2:
                                    # second step of a 4-k-tile quad: emit the
                                    # quad's 4 denominator matmuls adjacently
                                    # on disjoint 32-wide PE column groups so
                                    # they stream concurrently. The psum
                                    # has_written clear of start=True is
                                    # per-partition, so each col-tile's chain
                                    # opens with its own start.
                                    p_pt, p_rlo, p_offs = prev
                                    for kt in range(kt0 - 2, kt0 + 2):
                                        j = kt % 4
                                        h = kt % 2
                                        if kt < kt0:
                                            src, slo, so = p_pt, p_rlo[h], p_offs[h]
                                        else:
                                            src, slo, so = pt, rlo[h], offs[h]
                                        nc.tensor.matmul(
                                            ol[32 * j : 32 * j + 1, 512 + so :],
                                            lhsT=ones[:, :],
                                            rhs=src[:, slo : slo + 512 - so],
                                            start=(kt < 4),
                                            stop=(kt >= nkt - 4),
                                            skip_group_check=True,
                                            tile_position=(0, 32 * j),
                                        )

                            return consume

                        flush(LOOKAHEAD - 1)
                        pending.append(make_consumer())

                    # two evacuation copies into the per-bh staging (o and l
                    # quarters are not adjacent there); deferred like a step.
                    # One pair of per-bh DMAs flows out after the last group.
                    def make_evac(bh=bh, g=g, ol=ol, oln=oln):
                        def evac():
                            if bh == BH_PER_CORE - 1 and g == NG - 1:
                                nc.vector.tensor_copy(
                                    oln[:, g * 512 : (g + 1) * 512],
                                    ol[:, 0:512],
                                )
                                nc.vector.tensor_copy(
                                    oln[:, T + g * 512 : T + (g + 1) * 512],
                                    ol[:, 512:1024],
                                )
                            else:
                                nc.vector.tensor_copy(
                                    oln.rearrange("p (a c) -> p a c", a=2)[
                                        :, :, g * 512 : (g + 1) * 512
                                    ],
                                    ol.rearrange("p (a c) -> p a c", a=2),
                                )
                            if BATCH_OUT:
                                if g == NG - 1:
                                    nc.sync.dma_start(
                                        o_d.ap()[bh], oln[:, 0:T]
                                    )
                                    nc.sync.dma_start(
                                        l_d.ap()[bh],
                                        oln[0 : 32 * NL : 32, T : 2 * T],
                                    )
                            else:
                                nc.sync.dma_start(
                                    o_d.ap()[bh][:, g * 512 : (g + 1) * 512],
                                    oln[:, g * 512 : (g + 1) * 512],
                                )
                                nc.sync.dma_start(
                                    l_d.ap()[bh][:, g * 512 : (g + 1) * 512],
                                    oln[0 : 32 * NL : 32, T + g * 512 : T + (g + 1) * 512],
                                )

                        return evac

                    pending.append(make_evac())

            flush()

            if loop_n is not None:
                loop_cm.__exit__(None, None, None)

    nc.compile()
    return nc


def make_host_inputs(q, k, v, block_mask):
    """Split full inputs into 8 per-core input maps (4 (b,h) pairs each).

    Each pair's inputs are packed into one contiguous [128, XCOLS] fp16
    plane: [ Q^T | K^T | V(t%128 -> partition, 16, d) | m8 mask table ].
    """
    q, k, v = np.asarray(q), np.asarray(k), np.asarray(v)
    block_mask = np.asarray(block_mask)
    pairs = [(b, h) for b in range(B) for h in range(H)]
    kb_idx = np.arange(32)
    vis_causal = kb_idx[:, None] <= kb_idx[None, :]  # [kb, qb]
    c01 = (np.arange(128)[None, :] >= np.arange(128)[:, None]).astype(np.float16)

    in_maps = []
    for c in range(N_CORES):
        sel = pairs[c * BH_PER_CORE : (c + 1) * BH_PER_CORE]
        x = np.zeros((BH_PER_CORE, 128, XCOLS), np.float16)
        for i, (b, h) in enumerate(sel):
            kt_ = k[b, :, h, :].T
            qt_ = q[b, :, h, :].T
            x[i, :, XK03 : XK03 + 512] = kt_[:, 0:512]
            x[i, :, XK415 : XK415 + 1536] = kt_[:, 512:2048]
            x[i, :, XQ0 : XQ0 + 512] = qt_[:, 0:512]
            x[i, :, XQ13 : XQ13 + 1536] = qt_[:, 512:2048]
            # V: [t, d] -> [t % 128, t // 128, d]
            x[i, :, XV : XV + NT * 128] = (
                v[b, :, h, :].reshape(NT, 128, D).transpose(1, 0, 2).reshape(128, -1)
            )
            # m8 mask table; one 32-entry slot per step pair (kt0, kt0+1).
            # The kt0+1 half's entries are shifted down by its causal trim so
            # the slot matches the kernel's packed contiguous pt layout.
            vis = (block_mask[b, h].T & vis_causal).astype(np.float16)
            for g in range(NG):
                for kt0 in range(0, 4 * g + 4, 2):
                    s0 = STEP_OFF[g] + kt0
                    for hh in range(2):
                        kt = kt0 + hh
                        bt = 2 * max(0, kt - 4 * g)
                        if hh == 0:
                            p0, b_lo = s0 * 16, 0
                        else:
                            p0, b_lo = s0 * 16 + 16, bt
                        n = 8 - b_lo
                        for half in range(2):
                            kb = 2 * kt + half
                            x[
                                i,
                                half * 64 : (half + 1) * 64,
                                XM + p0 : XM + p0 + 2 * n,
                            ] = np.repeat(
                                vis[kb, 8 * g + b_lo : 8 * g + 8], 2
                            )[None, :]
        in_maps.append({"x": x, "c01": c01})
    return in_maps


_NC_CACHE = {}


def get_program():
    if "nc" not in _NC_CACHE:
        _NC_CACHE["nc"] = build_program()
    return _NC_CACHE["nc"]


def assemble_output(res, inputs=None):
    pairs = [(b, h) for b in range(B) for h in range(H)]
    out = np.zeros((B, T, H, D), np.float16)
    for c in range(N_CORES):
        sel = pairs[c * BH_PER_CORE : (c + 1) * BH_PER_CORE]
        oc = res.results[c]["o"]  # [bh, d, t] transposed-unnormalized fp32
        lr = res.results[c]["l"]  # [bh, NL, t] partial rows
        lc = lr.sum(axis=1)
        if lr.shape[1] > 1:
            # the single-quad g=0 group writes its denominator unpacked
            # (row 0 only); rows 1..3 hold stale bank data there
            lc[:, 0:512] = lr[:, 0, 0:512]
        for i, (b, h) in enumerate(sel):
            out[b, :, h, :] = (oc[i].T / lc[i][:, None]).astype(np.float16)
    return out


def kernel(q, k, v, block_mask, _trace=False):
    from concourse.bass_utils import run_bass_kernel_spmd

    nc = get_program()
    in_maps = make_host_inputs(q, k, v, block_mask)
    res = run_bass_kernel_spmd(
        nc, in_maps, core_ids=list(range(N_CORES)), trace=_trace
    )
    out = assemble_output(res)
    if _trace:
        return out, res
    return out

